# revision 1
# baseline (speedup 1.0000x reference)
"""BloomBlock on 8 TRN2 NeuronCores.

Strategy (no device collectives — they have a ~60-100us floor on this part):
  * Launch 1 (L1): data-parallel over tokens. Each core owns 2 query blocks
    of 128 tokens (blocks i and 15-i — balances causal attention work).
    Computes LN1 (folded into weights on host), then Q^T,K^T (feature-major)
    and V (token-major) for its own 256 tokens, all in bf16.
  * Host: gathers K/V from all cores, builds per-core padded/permuted key
    layouts (slot 0 = block-A diagonal chunk, slot 1 = block-B diagonal
    chunk, then the remaining causal prefix, then zero pads). Masking and
    alibi are folded into per-(head,slot) exp-bias columns; out-of-prefix
    and pad keys get -1e9 so exp() underflows to exactly 0.
  * Launch 2 (L2): attention in transposed-score layout (keys on
    partitions, queries on free dim — softmax sum arrives free via an
    appended ones-column on V), then dense + residual, LN2, MLP, residual.
    All matmuls bf16 with fp32 PSUM accumulation; residual path fp32.
"""

import os
from contextlib import ExitStack

import ml_dtypes
import numpy as np

import concourse.bass as bass
import concourse.tile as tile
from concourse import bacc, mybir
from concourse.masks import make_identity

BF16 = mybir.dt.bfloat16
F32 = mybir.dt.float32
F32R = mybir.dt.float32r
NBF = ml_dtypes.bfloat16

S, H, NH, HD = 2048, 1024, 16, 64
NCORE = 8
QB = 128          # query/key chunk size
SC = 2 * QB       # tokens per core
NSLOT = 16        # padded key-chunk slots per core
NHC = H // 128    # hidden chunks
EPS = 1e-5
NORM = float(np.sqrt(HD))  # 8.0 (LAYER_NUMBER = 1)
NEG = -1e9


def _blocks(i):
    return (i, 15 - i)


def _slots(i):
    """Key-chunk id per slot for core i (None = zero pad). Diagonal chunks
    sit at fixed slots 14 (block A) and 15 (block B) so the post-exp mask
    multiplies land at the tail of the ctx accumulation chain."""
    a, b = _blocks(i)
    rest = [c for c in range(b + 1) if c not in (a, b)]
    s = rest + [None] * (NSLOT - 2 - len(rest))
    return s + [a, b]


# ----------------------------------------------------------------------------
# device kernels
# ----------------------------------------------------------------------------

def _layernorm_tiles(nc, tc, pools, x_tiles, out_all, ident, epst):
    """x_tiles: list of 2 [128, H] f32 sbuf tiles (token-major).
    Writes xhat^T (feature-major, bf16) into out_all [128, NHC*SC]
    (chunk c at cols [c*SC:(c+1)*SC], token-group t at [c*SC+t*128 ...])."""
    stat, tp_ps, work = pools
    for t, xt in enumerate(x_tiles):
        st = stat.tile([128, 2, 6], F32, tag="bnst")
        nc.vector.bn_stats(out=st[:, 0, :], in_=xt[:, 0:512])
        nc.vector.bn_stats(out=st[:, 1, :], in_=xt[:, 512:1024])
        mv = stat.tile([128, 2], F32, tag="bnmv")
        nc.vector.bn_aggr(out=mv, in_=st)
        rstd = stat.tile([128, 1], F32, tag="rstd")
        nc.scalar.activation(out=rstd, in_=mv[:, 1:2],
                             func=mybir.ActivationFunctionType.Sqrt,
                             bias=epst, scale=1.0)
        nc.vector.reciprocal(out=rstd, in_=rstd)
        xh = work.tile([128, H], BF16, tag="xhat")
        nc.vector.tensor_scalar(out=xh, in0=xt, scalar1=mv[:, 0:1], scalar2=rstd,
                                op0=mybir.AluOpType.subtract,
                                op1=mybir.AluOpType.mult)
        for c in range(NHC):
            pst = tp_ps.tile([128, 128], BF16, tag="tp")
            nc.tensor.transpose(pst, xh[:, c * 128:(c + 1) * 128], ident)
            nc.vector.tensor_copy(
                out=out_all[c][:, t * 128:(t + 1) * 128],
                in_=pst)


def build_l1():
    nc = bacc.Bacc("TRN2", target_bir_lowering=False, debug=False,
                   num_devices=NCORE)
    x = nc.dram_tensor("x", [SC, H], F32, kind="ExternalInput")
    wqkT = nc.dram_tensor("wqkT", [H, 2 * H], BF16, kind="ExternalInput")
    wvT = nc.dram_tensor("wvT", [H, H], BF16, kind="ExternalInput")
    bqk = nc.dram_tensor("bqk", [128, 16], F32, kind="ExternalInput")
    bv = nc.dram_tensor("bv", [1, H], F32, kind="ExternalInput")
    qkT = nc.dram_tensor("qkT", [2 * H, SC], BF16, kind="ExternalOutput")
    vtm = nc.dram_tensor("vtm", [SC, H], BF16, kind="ExternalOutput")

    with tile.TileContext(nc) as tc, ExitStack() as ctx:
        singles = ctx.enter_context(tc.tile_pool(name="singles", bufs=1))
        stat = ctx.enter_context(tc.tile_pool(name="stat", bufs=2))
        work = ctx.enter_context(tc.tile_pool(name="work", bufs=2))
        tp_ps = ctx.enter_context(tc.tile_pool(name="tp_ps", bufs=1, space="PSUM"))
        mm_ps = ctx.enter_context(tc.tile_pool(name="mm_ps", bufs=3, space="PSUM"))
        opool = ctx.enter_context(tc.tile_pool(name="opool", bufs=6))

        ident = singles.tile([128, 128], BF16)
        make_identity(nc, ident)
        epst = singles.tile([128, 1], F32)
        nc.vector.memset(epst, EPS)
        x_tiles = []
        for t in range(2):
            xt = singles.tile([128, H], F32, tag=f"x{t}")
            nc.sync.dma_start(out=xt, in_=x.ap()[t * 128:(t + 1) * 128, :])
            x_tiles.append(xt)
        xhatT = []
        for c in range(NHC):
            xc = singles.tile([128, SC], BF16, tag=f"xhatT{c}")
            xhatT.append(xc)
        _layernorm_tiles(nc, tc, (stat, tp_ps, work), x_tiles, xhatT, ident, epst)

        wqk_sb = []
        for c in range(NHC):
            wt = singles.tile([128, 2 * H], BF16, tag=f"wqk{c}")
            nc.sync.dma_start(out=wt, in_=wqkT.ap()[c * 128:(c + 1) * 128, :])
            wqk_sb.append(wt)
        wv_sb = []
        for c in range(NHC):
            wt = singles.tile([128, H], BF16, tag=f"wv{c}")
            nc.sync.dma_start(out=wt, in_=wvT.ap()[c * 128:(c + 1) * 128, :])
            wv_sb.append(wt)
        bqk_sb = singles.tile([128, 16], F32)
        nc.sync.dma_start(out=bqk_sb, in_=bqk.ap())
        bv_sb = singles.tile([128, H], F32)
        nc.sync.dma_start(out=bv_sb, in_=bv.ap().to_broadcast([128, H]))

        # Q^T / K^T, feature-major [2H, SC]
        for m in range(16):
            ps = mm_ps.tile([128, SC], F32, tag="psqk", bufs=4)
            for c in range(NHC):
                nc.tensor.matmul(
                    ps,
                    lhsT=wqk_sb[c][:, m * 128:(m + 1) * 128],
                    rhs=xhatT[c],
                    start=(c == 0), stop=(c == NHC - 1))
            ot = opool.tile([128, SC], BF16, tag="oqk")
            nc.vector.tensor_scalar_add(out=ot, in0=ps, scalar1=bqk_sb[:, m:m + 1])
            nc.sync.dma_start(out=qkT.ap()[m * 128:(m + 1) * 128, :], in_=ot)

        # V, token-major [SC, H]
        for t in range(2):
            for n in range(2):
                ps = mm_ps.tile([128, 512], F32, tag="psv")
                for c in range(NHC):
                    nc.tensor.matmul(
                        ps,
                        lhsT=xhatT[c][:, t * 128:(t + 1) * 128],
                        rhs=wv_sb[c][:, n * 512:(n + 1) * 512],
                        start=(c == 0), stop=(c == NHC - 1))
                vt = opool.tile([128, 512], BF16, tag="ov")
                nc.vector.tensor_add(out=vt, in0=ps, in1=bv_sb[:, n * 512:(n + 1) * 512])
                nc.sync.dma_start(
                    out=vtm.ap()[t * 128:(t + 1) * 128, n * 512:(n + 1) * 512],
                    in_=vt)
    nc.compile()
    return nc


def build_l2():
    nc = bacc.Bacc("TRN2", target_bir_lowering=False, debug=False,
                   num_devices=NCORE)
    qaug = nc.dram_tensor("qaug", [NH, 66, SC], BF16, kind="ExternalInput")
    kaug = nc.dram_tensor("kaug", [NH, 66, NSLOT * QB], BF16, kind="ExternalInput")
    vaug = nc.dram_tensor("vaug", [NSLOT, QB, NH * 65], BF16, kind="ExternalInput")
    binm = nc.dram_tensor("binm", [QB, QB], BF16, kind="ExternalInput")
    xres = nc.dram_tensor("xres", [SC, H], F32, kind="ExternalInput")
    dwT = nc.dram_tensor("dwT", [H, H], BF16, kind="ExternalInput")
    fc1T = nc.dram_tensor("fc1T", [H, 4 * H], BF16, kind="ExternalInput")
    b1 = nc.dram_tensor("b1", [128, 32], F32, kind="ExternalInput")
    fc2T = nc.dram_tensor("fc2T", [4 * H, H], BF16, kind="ExternalInput")
    b2 = nc.dram_tensor("b2", [1, H], F32, kind="ExternalInput")
    out = nc.dram_tensor("out", [SC, H], F32, kind="ExternalOutput")

    with tile.TileContext(nc) as tc, ExitStack() as ctx:
        singles = ctx.enter_context(tc.tile_pool(name="singles", bufs=1))
        stat = ctx.enter_context(tc.tile_pool(name="stat", bufs=2))
        work = ctx.enter_context(tc.tile_pool(name="work", bufs=2))
        tp_ps = ctx.enter_context(tc.tile_pool(name="tp_ps", bufs=2, space="PSUM"))

        ident = singles.tile([128, 128], BF16)
        make_identity(nc, ident)
        epst = singles.tile([128, 1], F32)
        nc.vector.memset(epst, EPS)
        bm = singles.tile([QB, QB], BF16)
        nc.sync.dma_start(out=bm, in_=binm.ap())
        qaug_t = []
        for h in range(NH):
            qt = singles.tile([66, SC], BF16, tag=f"qaug{h}")
            nc.sync.dma_start(out=qt, in_=qaug.ap()[h])
            qaug_t.append(qt)
        ones_col = singles.tile([1, 64], F32)
        nc.vector.memset(ones_col, 1.0)

        ctxT = []
        for c in range(NHC):
            ct = singles.tile([128, SC], BF16, tag=f"ctxT{c}")
            ctxT.append(ct)
        attn_t = []
        for t in range(2):
            at = singles.tile([128, H], F32, tag=f"attn{t}")
            attn_t.append(at)

        # ------------------- attention -------------------
        _phase = os.environ.get("BLOOM_PHASE", "all")
        NB = int(os.environ.get("BLOOM_NB", "4"))  # key-chunk slots per exp batch
        if _phase in ("all", "attn"):
          _scb = int(os.environ.get("BLOOM_SCB", "3"))
          _ppb = int(os.environ.get("BLOOM_PPB", "8"))
          _ctb = int(os.environ.get("BLOOM_CTB", "1"))
          _rcb = int(os.environ.get("BLOOM_RCB", "1"))
          with tc.tile_pool(name="attn", bufs=1) as apool, \
             tc.tile_pool(name="probs", bufs=_ppb) as ppool, \
             tc.tile_pool(name="sc_ps", bufs=_scb, space="PSUM") as sc_ps, \
             tc.tile_pool(name="ctx_ps", bufs=_ctb, space="PSUM") as ctx_ps:

            # interleave K/V loads: the first ctx matmuls need the early V
            # slots, so they must not queue behind all 16 kaug streams
            kaug_t = []
            va_sb = []
            for i in range(NSLOT):
                kt = apool.tile([66, NSLOT * QB], BF16, tag=f"kaug{i}")
                nc.sync.dma_start(out=kt, in_=kaug.ap()[i])
                kaug_t.append(kt)
                vt = apool.tile([128, NH * 65], BF16, tag=f"va{i}")
                nc.sync.dma_start(out=vt, in_=vaug.ap()[i])
                va_sb.append(vt)

            for h in range(NH):
                c = h // 2
                p0 = 64 * (h % 2)
                pctx = ctx_ps.tile([65, SC], F32, tag="pctx")
                for b in range(NSLOT // NB):
                    psb = sc_ps.tile([128, NB * SC], F32, tag="psb")
                    for j in range(NB):
                        s = b * NB + j
                        nc.tensor.matmul(
                            psb[:, j * SC:(j + 1) * SC],
                            lhsT=kaug_t[h][:, s * QB:(s + 1) * QB],
                            rhs=qaug_t[h],
                            start=True, stop=True)
                    probs = ppool.tile([128, NB * SC], BF16, tag="probs")
                    nc.scalar.activation(out=probs, in_=psb,
                                         func=mybir.ActivationFunctionType.Exp,
                                         bias=0.0, scale=1.0)
                    if b == NSLOT // NB - 1:
                        # diagonal-chunk causal masks: slot 14 masks the
                        # A-half, slot 15 the B-half (binary mult, post-exp)
                        j14 = (NSLOT - 2) % NB
                        nc.vector.tensor_mul(
                            out=probs[:, j14 * SC: j14 * SC + QB],
                            in0=probs[:, j14 * SC: j14 * SC + QB], in1=bm)
                        j15 = (NSLOT - 1) % NB
                        nc.vector.tensor_mul(
                            out=probs[:, j15 * SC + QB:(j15 + 1) * SC],
                            in0=probs[:, j15 * SC + QB:(j15 + 1) * SC], in1=bm)
                    for j in range(NB):
                        s = b * NB + j
                        nc.tensor.matmul(
                            pctx,
                            lhsT=va_sb[s][:, h * 65:(h + 1) * 65],
                            rhs=probs[:, j * SC:(j + 1) * SC],
                            start=(s == 0), stop=(s == NSLOT - 1))
                recip = work.tile([1, SC], F32, tag="recip")
                nc.vector.reciprocal(out=recip, in_=pctx[64:65, :])
                prec = ctx_ps.tile([64, SC], F32, tag="prec", bufs=_rcb)
                nc.tensor.matmul(prec, lhsT=ones_col, rhs=recip,
                                 start=True, stop=True)
                recb = work.tile([64, SC], F32, tag="recb")
                nc.vector.tensor_copy(out=recb, in_=prec)
                nc.vector.tensor_mul(
                    out=ctxT[c][p0:p0 + 64, :],
                    in0=pctx[0:64, :], in1=recb)

        # ------------------- dense + residual + LN2 + MLP -------------------
        if _phase in ("all", "rest"):
          b1_sb = singles.tile([128, 32], F32)
          nc.sync.dma_start(out=b1_sb, in_=b1.ap())
          b2_sb = singles.tile([128, H], F32)
          nc.sync.dma_start(out=b2_sb, in_=b2.ap().to_broadcast([128, H]))
          with tc.tile_pool(name="f1pool", bufs=1) as f1p:
            with tc.tile_pool(name="dw", bufs=1) as dwp, \
                 tc.tile_pool(name="mm2_ps", bufs=4, space="PSUM") as mm_ps:
              dw_sb = []
              for c in range(NHC):
                wt = dwp.tile([128, H], BF16, tag=f"dw{c}")
                nc.sync.dma_start(out=wt, in_=dwT.ap()[c * 128:(c + 1) * 128, :])
                dw_sb.append(wt)
              xres_t = []
              for t in range(2):
                xt = dwp.tile([128, H], F32, tag=f"xres{t}")
                nc.sync.dma_start(out=xt, in_=xres.ap()[t * 128:(t + 1) * 128, :])
                xres_t.append(xt)
              fc1_sb = []
              for c in range(NHC):
                wt = f1p.tile([128, 4 * H], BF16, tag=f"fc1{c}")
                nc.sync.dma_start(out=wt,
                                  in_=fc1T.ap()[c * 128:(c + 1) * 128, :])
                fc1_sb.append(wt)
              for t in range(2):
                for n in range(2):
                    ps = mm_ps.tile([128, 512], F32, tag="psd")
                    for c in range(NHC):
                        nc.tensor.matmul(
                            ps,
                            lhsT=ctxT[c][:, t * 128:(t + 1) * 128],
                            rhs=dw_sb[c][:, n * 512:(n + 1) * 512],
                            start=(c == 0), stop=(c == NHC - 1))
                    sl = slice(n * 512, (n + 1) * 512)
                    nc.vector.tensor_add(out=attn_t[t][:, sl], in0=ps,
                                         in1=xres_t[t][:, sl])

            xh2T = []
            for c in range(NHC):
                xc = singles.tile([128, SC], BF16, tag=f"xh2T{c}")
                xh2T.append(xc)
            with tc.tile_pool(name="tp2_ps", bufs=2, space="PSUM") as tp2_ps:
                _layernorm_tiles(nc, tc, (stat, tp2_ps, work), attn_t, xh2T,
                                 ident, epst)

            with tc.tile_pool(name="hpool", bufs=1) as hp, \
                 tc.tile_pool(name="mm3_ps", bufs=3, space="PSUM") as mm_ps:
              hT = []
              for m in range(32):
                  htile = hp.tile([128, SC], BF16, tag=f"hT{m}")
                  hT.append(htile)
              for m in range(32):
                ps = mm_ps.tile([128, SC], F32, tag="psf1", bufs=5)
                for c in range(NHC):
                    nc.tensor.matmul(
                        ps,
                        lhsT=fc1_sb[c][:, m * 128:(m + 1) * 128],
                        rhs=xh2T[c],
                        start=(c == 0), stop=(c == NHC - 1))
                if os.environ.get("BLOOM_SIM") or os.environ.get("BLOOM_GELU_CHAIN"):
                    # bloom gelu: u*0.5*(1+tanh(0.79788456*u*(1+0.044715*u^2)))
                    # (explicit chain — CoreSim has no Gelu_apprx_tanh)
                    u = work.tile([128, SC], F32, tag="gelu_u")
                    nc.vector.tensor_scalar_add(out=u, in0=ps,
                                                scalar1=b1_sb[:, m:m + 1])
                    s2 = work.tile([128, SC], F32, tag="gelu_s")
                    nc.vector.tensor_mul(out=s2, in0=u, in1=u)
                    nc.vector.tensor_scalar(out=s2, in0=s2,
                                            scalar1=0.035677408145115,
                                            scalar2=0.7978845608028654,
                                            op0=mybir.AluOpType.mult,
                                            op1=mybir.AluOpType.add)
                    nc.vector.tensor_mul(out=s2, in0=s2, in1=u)
                    nc.scalar.activation(out=s2, in_=s2,
                                         func=mybir.ActivationFunctionType.Tanh,
                                         bias=0.0, scale=1.0)
                    nc.vector.tensor_scalar(out=s2, in0=s2, scalar1=1.0,
                                            scalar2=0.5,
                                            op0=mybir.AluOpType.add,
                                            op1=mybir.AluOpType.mult)
                    nc.vector.tensor_mul(out=hT[m], in0=s2, in1=u)
                else:
                    nc.scalar.activation(
                        out=hT[m], in_=ps,
                        func=mybir.ActivationFunctionType.Gelu_apprx_tanh,
                        bias=b1_sb[:, m:m + 1], scale=1.0)

              fc2_sb = []
              for c2 in range(32):
                  wt = hp.tile([128, H], BF16, tag=f"fc2{c2}")
                  nc.sync.dma_start(out=wt,
                                    in_=fc2T.ap()[c2 * 128:(c2 + 1) * 128, :])
                  fc2_sb.append(wt)
              for t in range(2):
                  for n in range(2):
                      ps = mm_ps.tile([128, 512], F32, tag="psf2")
                      for c2 in range(32):
                          nc.tensor.matmul(
                              ps,
                              lhsT=hT[c2][:, t * 128:(t + 1) * 128],
                              rhs=fc2_sb[c2][:, n * 512:(n + 1) * 512],
                              start=(c2 == 0), stop=(c2 == 31))
                      sl = slice(n * 512, (n + 1) * 512)
                      ot = work.tile([128, 512], F32, tag="oto")
                      nc.vector.tensor_add(out=ot, in0=ps, in1=attn_t[t][:, sl])
                      nc.vector.tensor_add(out=ot, in0=ot, in1=b2_sb[:, sl])
                      nc.sync.dma_start(out=out.ap()[t * 128:(t + 1) * 128, sl], in_=ot)
    nc.compile()
    return nc


# ----------------------------------------------------------------------------
# host orchestration
# ----------------------------------------------------------------------------

_NC_CACHE = {}


def _get_nc(name):
    if name not in _NC_CACHE:
        _NC_CACHE[name] = build_l1() if name == "l1" else build_l2()
    return _NC_CACHE[name]


def _run(nc, in_maps):
    if os.environ.get("BLOOM_SIM"):
        from concourse.bass_interp import CoreSim
        results = []
        for m in in_maps:
            sim = CoreSim(nc, trace=False)
            for k, v in m.items():
                sim.tensor(k)[:] = v
            sim.simulate(check_with_hw=False)
            outs = {}
            for alloc in nc.m.functions[0].allocations:
                if getattr(alloc, "kind", None) == "ExternalOutput":
                    nm = alloc.memorylocations[0].name
                    outs[nm] = np.array(sim.tensor(nm))
            results.append(outs)
        return results
    from concourse.bass_utils import run_bass_kernel_spmd
    res = run_bass_kernel_spmd(nc, in_maps, core_ids=list(range(NCORE)))
    return res.results


def _prep_weights(ln1_g, ln1_b, qkv_w, qkv_b, dense_w, dense_b,
                  ln2_g, ln2_b, fc1_w, fc1_b, fc2_w, fc2_b):
    qkv_w = np.asarray(qkv_w, np.float32)
    qkv_b = np.asarray(qkv_b, np.float32)
    w_eff = qkv_w * np.asarray(ln1_g, np.float32)[None, :]
    b_eff = qkv_b + qkv_w @ np.asarray(ln1_b, np.float32)
    w3 = w_eff.reshape(NH, 3 * HD, H)
    b3 = b_eff.reshape(NH, 3 * HD)
    wq = w3[:, :HD, :] / NORM
    wk = w3[:, HD:2 * HD, :]
    wv = w3[:, 2 * HD:, :]
    bq = b3[:, :HD] / NORM
    bk = b3[:, HD:2 * HD]
    bv = b3[:, 2 * HD:]
    wqk = np.concatenate([wq.reshape(H, H), wk.reshape(H, H)], 0)  # [2H, H]
    wqkT = np.ascontiguousarray(wqk.T).astype(NBF)                  # [H, 2H]
    wvT = np.ascontiguousarray(wv.reshape(H, H).T).astype(NBF)      # [H, H]
    bqk = np.concatenate([bq.reshape(H), bk.reshape(H)])            # [2H]
    bqk_t = np.ascontiguousarray(bqk.reshape(16, 128).T).astype(np.float32)
    bv_r = np.ascontiguousarray(bv.reshape(1, H)).astype(np.float32)

    dwT = np.ascontiguousarray(np.asarray(dense_w, np.float32).T).astype(NBF)
    db_r = np.asarray(dense_b, np.float32).reshape(1, H)

    f1_eff = np.asarray(fc1_w, np.float32) * np.asarray(ln2_g, np.float32)[None, :]
    b1_eff = np.asarray(fc1_b, np.float32) + np.asarray(fc1_w, np.float32) @ np.asarray(ln2_b, np.float32)
    fc1T = np.ascontiguousarray(f1_eff.T).astype(NBF)               # [H, 4H]
    b1_t = np.ascontiguousarray(b1_eff.reshape(32, 128).T).astype(np.float32)
    fc2T = np.ascontiguousarray(np.asarray(fc2_w, np.float32).T).astype(NBF)
    b2_r = np.asarray(fc2_b, np.float32).reshape(1, H)
    return dict(wqkT=wqkT, wvT=wvT, bqk=bqk_t, bv=bv_r, dwT=dwT, db=db_r,
                fc1T=fc1T, b1=b1_t, fc2T=fc2T, b2=b2_r)


def _tri_mask():
    k = np.arange(QB)[:, None]
    q = np.arange(QB)[None, :]
    return np.where(k <= q, 1.0, 0.0).astype(NBF)   # [k, q] allowed k<=q


def kernel(hidden_states, attention_mask, alibi,
           ln1_g, ln1_b, qkv_w, qkv_b, dense_w, dense_b,
           ln2_g, ln2_b, fc1_w, fc1_b, fc2_w, fc2_b):
    X = np.asarray(hidden_states, np.float32).reshape(S, H)
    alibi_np = np.asarray(alibi, np.float32).reshape(NH, S)
    W = _prep_weights(ln1_g, ln1_b, qkv_w, qkv_b, dense_w, dense_b,
                      ln2_g, ln2_b, fc1_w, fc1_b, fc2_w, fc2_b)

    # ---------------- L1 ----------------
    nc1 = _get_nc("l1")
    in1 = []
    xcore = []
    for i in range(NCORE):
        a, b = _blocks(i)
        xi = np.ascontiguousarray(
            np.concatenate([X[a * QB:(a + 1) * QB], X[b * QB:(b + 1) * QB]], 0))
        xcore.append(xi)
        in1.append(dict(x=xi, wqkT=W["wqkT"], wvT=W["wvT"],
                        bqk=W["bqk"], bv=W["bv"]))
    r1 = _run(nc1, in1)

    # ---------------- host reshuffle ----------------
    KT = np.zeros((H, S), NBF)
    V = np.zeros((S, H), NBF)
    qT_core = []
    for i in range(NCORE):
        a, b = _blocks(i)
        qkT_i = r1[i]["qkT"]
        vtm_i = r1[i]["vtm"]
        qT_core.append(qkT_i[:H])
        KT[:, a * QB:(a + 1) * QB] = qkT_i[H:, :QB]
        KT[:, b * QB:(b + 1) * QB] = qkT_i[H:, QB:]
        V[a * QB:(a + 1) * QB] = vtm_i[:QB]
        V[b * QB:(b + 1) * QB] = vtm_i[QB:]

    binm = _tri_mask()
    nc2 = _get_nc("l2")
    in2 = []
    for i in range(NCORE):
        a, bq_ = _blocks(i)
        slots = _slots(i)
        # qaug: per head [66, SC]: rows 0:64 = q^T, row 64 = 1 on A-half,
        # row 65 = 1 on B-half
        qaug = np.zeros((NH, 66, SC), NBF)
        qf = qT_core[i].astype(np.float32).reshape(NH, HD, SC)
        qaug[:, :HD, :] = qf
        qaug[:, HD, :QB] = 1.0
        qaug[:, HD + 1, QB:] = 1.0
        # kaug: per head [66, NSLOT*QB]: rows 0:64 = k^T (permuted chunks),
        # row 64 = alibi + A-prefix mask, row 65 = alibi + B-prefix mask
        kaug = np.zeros((NH, 66, NSLOT * QB), np.float32)
        kaug[:, HD, :] = NEG
        kaug[:, HD + 1, :] = NEG
        KTf = KT.astype(np.float32).reshape(NH, HD, S)
        for s, c in enumerate(slots):
            if c is None:
                continue
            ck = slice(c * QB, (c + 1) * QB)
            sk = slice(s * QB, (s + 1) * QB)
            kaug[:, :HD, sk] = KTf[:, :, ck]
            av = alibi_np[:, ck]
            kaug[:, HD, sk] = av if c <= a else NEG
            kaug[:, HD + 1, sk] = av
        vaug = np.zeros((NSLOT, QB, NH * 65), NBF)
        for s, c in enumerate(slots):
            if c is None:
                continue
            vs = V[c * QB:(c + 1) * QB].astype(np.float32)
            for h in range(NH):
                vaug[s, :, h * 65: h * 65 + HD] = vs[:, h * HD:(h + 1) * HD]
                vaug[s, :, h * 65 + HD] = 1.0
        in2.append(dict(qaug=qaug, kaug=kaug.astype(NBF), vaug=vaug,
                        binm=binm, xres=xcore[i] + W["db"],
                        dwT=W["dwT"], fc1T=W["fc1T"], b1=W["b1"],
                        fc2T=W["fc2T"], b2=W["b2"]))
    r2 = _run(nc2, in2)

    out = np.zeros((S, H), np.float32)
    for i in range(NCORE):
        a, b = _blocks(i)
        oi = r2[i]["out"]
        out[a * QB:(a + 1) * QB] = oi[:QB]
        out[b * QB:(b + 1) * QB] = oi[QB:]
    return out.reshape(1, S, H)

